# revision 15
# baseline (speedup 1.0000x reference)
"""DNN_Beamformer (MVDR + attention reference) on 8 Trainium2 NeuronCores.

B-sharded: one batch element per core, full inputs in / full output out.
Two Bass kernel launches per call:
  K1: masked cross-channel PSDs for both masks on the PE array with T on the
      contraction axis.  Mask channel sums run as an add-tree split across
      DVE and GpSimd; the f->t transpose and the ri-duplication are fused
      into ONE PE matmul against a duplicated identity (D[i, 2i]=D[i,2i+1]=1)
      accumulating nothing; xw (mask-weighted data) is built with full-F
      contiguous tensor_tensor ops split across DVE and GpSimd.
      K1 does NOT write a data copy - K2 re-reads the original f32 data.
  host middle: PSD normalization, attention reference (MLP + softmax),
      8x8 complex inverse, MVDR weights -> beamforming vector bf (tiny).
  K2: beamforming apply from the ORIGINAL f32 data: linear loads, bf16 cast
      on ACT/DVE, PE transposes to f-major, then rotation matmuls with the
      block-diagonal weights as the REUSED stationary operand (contraction
      over (f,ri), accumulation over channels in PSUM).  Output lands
      f-major [(f,ri), T]; the host transposes it back (free).

When BEAM_TRACE is set, each kernel runs twice: once traced (timing only -
NTFF profiling corrupts concurrently running cores) and once clean for the
actual outputs.

If anything in the device path fails, falls back to a pure-numpy pipeline
so the output is always correct.
"""

import os
import sys

import numpy as np

sys.path.insert(0, '/opt/trn_rl_repo')

B, T, C, F = 8, 1024, 8, 513
A = 320
EPS_MASK = 1e-6
EPS_PSD = 1e-15
EPS_MVDR = 1e-15
SCALING = 2.0

P = 128
NT = T // P                      # 8 t-tiles
F_TILES = [(i * P, min(P, F - i * P)) for i in range((F + P - 1) // P)]  # 4x128 + 1
PSUM_W = 16                      # one (32,16) psd slot per (f)
BANK = 512                       # PSUM bank free f32


def _off(g):
    return (g // 32) * BANK + (g % 32) * PSUM_W


def _build_psd_kernel():
    import concourse.bacc as bacc
    import concourse.mybir as mybir
    from concourse.tile import TileContext
    from concourse import masks as cmasks

    nc = bacc.Bacc(None, target_bir_lowering=False)
    fp = mybir.dt.float32
    bh = mybir.dt.bfloat16
    data_d = nc.dram_tensor("data", [T, C, F, 2], fp, kind="ExternalInput")
    ms_d = nc.dram_tensor("ms", [F, C, T], fp, kind="ExternalInput")
    mn_d = nc.dram_tensor("mn", [F, C, T], fp, kind="ExternalInput")
    psd_d = nc.dram_tensor("psd_raw", [P, 5 * BANK], fp, kind="ExternalOutput")
    msum_d = nc.dram_tensor("msum", [P, 10], fp, kind="ExternalOutput")

    mult = mybir.AluOpType.mult
    add = mybir.AluOpType.add

    with TileContext(nc) as tc:
        with tc.tile_pool(name="const", bufs=1) as cpool, \
             tc.tile_pool(name="big", bufs=2) as bigpool, \
             tc.tile_pool(name="mf", bufs=2) as mfpool, \
             tc.tile_pool(name="mb", bufs=2) as mbpool, \
             tc.tile_pool(name="mt", bufs=1) as mtpool, \
             tc.tile_pool(name="tp", bufs=2, space="PSUM") as tppool, \
             tc.tile_pool(name="bfc", bufs=3) as bfpool, \
             tc.tile_pool(name="xw", bufs=2) as xwpool, \
             tc.tile_pool(name="psum", bufs=1, space="PSUM") as pspool, \
             tc.tile_pool(name="out", bufs=1) as opool:

            identb = cpool.tile([P, P], bh)
            cmasks.make_identity(nc, identb[:])
            # D: duplicated identity, D[i, 2i] = D[i, 2i+1] = 1
            dup = cpool.tile([P, 2 * P], bh, tag="dup")
            nc.vector.memset(dup[:], 0.0)
            nc.vector.tensor_copy(dup[:, 0:2 * P:2], identb[:])
            nc.vector.tensor_copy(dup[:, 1:2 * P:2], identb[:])

            msum_t = opool.tile([P, 10], fp, tag="msum")
            nc.vector.memset(msum_t[:], 0.0)

            # --- phase A: mask channel sums (DVE/GpSimd add tree),
            #     then fused transpose+ri-dup on PE ---
            mtds = [[mtpool.tile([P, F, 2], bh, tag=f"mtd{mk}_{it}",
                                 name=f"mtd{mk}_{it}")
                     for it in range(NT)] for mk in range(2)]
            TH2 = T // 2
            for mk, md in enumerate((ms_d, mn_d)):
                for ft, (f0, pf) in enumerate(F_TILES):
                    m = mfpool.tile([P, T], fp, tag="m")
                    for th in range(2):
                        mtile = bigpool.tile([P, C, TH2], fp, tag="big",
                                             name="mtile")
                        nc.sync.dma_start(
                            mtile[:pf], md[f0:f0 + pf, :,
                                           th * TH2:(th + 1) * TH2])
                        nc.vector.tensor_tensor(mtile[:pf, 0], mtile[:pf, 0],
                                                mtile[:pf, 1], add)
                        nc.vector.tensor_tensor(mtile[:pf, 2], mtile[:pf, 2],
                                                mtile[:pf, 3], add)
                        nc.gpsimd.tensor_tensor(mtile[:pf, 4], mtile[:pf, 4],
                                                mtile[:pf, 5], add)
                        nc.gpsimd.tensor_tensor(mtile[:pf, 6], mtile[:pf, 6],
                                                mtile[:pf, 7], add)
                        nc.vector.tensor_tensor(mtile[:pf, 0], mtile[:pf, 0],
                                                mtile[:pf, 2], add)
                        nc.gpsimd.tensor_tensor(mtile[:pf, 4], mtile[:pf, 4],
                                                mtile[:pf, 6], add)
                        nc.vector.tensor_tensor(
                            m[:pf, th * TH2:(th + 1) * TH2],
                            mtile[:pf, 0], mtile[:pf, 4], add)
                    nc.vector.tensor_reduce(
                        msum_t[0:pf, mk * 5 + ft:mk * 5 + ft + 1], m[:pf],
                        axis=mybir.AxisListType.X, op=add)
                    mb = mbpool.tile([P, T], bh, tag="mb")
                    nc.scalar.copy(mb[:pf], m[:pf])
                    for it in range(NT):
                        tp = tppool.tile([P, 2 * P], bh, tag="tp")
                        # out[t, (f,ri)] = sum_f mb[f, t] * D[f, (f,ri)]
                        nc.tensor.transpose(tp[:, :2 * pf],
                                            mb[:pf, it * P:(it + 1) * P],
                                            dup[:pf, :2 * pf])
                        nc.scalar.copy(mtds[mk][it][:, f0:f0 + pf, :],
                                       tp[:, :2 * pf])
            nc.sync.dma_start(msum_d[:], msum_t[:])

            # --- phase B: data load + cast + xw + PSD matmuls ---
            psd_ps = pspool.tile([P, 5 * BANK], fp, tag="psd")
            # Replay safety: profiling can re-execute the NEFF with PSUM
            # has_written bits persisting from the prior run.
            nc.vector.memset(psd_ps[:], 0.0)
            for it in range(NT):
                bfc = bfpool.tile([P, C, F, 2], bh, tag="bfc", name="bfc")
                # SWDGE casting DMA (f32 HBM -> bf16 SBUF); runs on the
                # gpsimd queue in parallel with the HWDGE mask stream.
                nc.gpsimd.dma_start(bfc[:], data_d[it * P:(it + 1) * P])
                # xw rows ordered r = mk*16 + 2c + ri as the decode expects.
                xw = xwpool.tile([P, F, 32], bh, tag="xw")
                for mk in range(2):
                    for c in range(C):
                        r0 = mk * 16 + 2 * c
                        eng = (nc.gpsimd if (mk * 8 + c) % 16 in
                               (1, 3, 6, 8, 11, 13, 14) else nc.vector)
                        eng.tensor_tensor(
                            xw[:, :, r0:r0 + 2],
                            bfc[:, c],
                            mtds[mk][it][:], mult)
                for f in range(F):
                    g, j = f // 4, f % 4
                    o = _off(g)
                    nc.tensor.matmul(
                        psd_ps[32 * j:32 * (j + 1), o:o + PSUM_W],
                        xw[:, f, :],
                        bfc[:, :, f, :],
                        # start=True clears has_written for the WHOLE
                        # bank -> only the first matmul per bank sets it
                        start=(it == 0 and f % 128 == 0),
                        stop=(it == NT - 1),
                        skip_group_check=True,
                        tile_position=(0, 32 * j))

            for bk in range(5):
                psd_sb = opool.tile([P, BANK], fp, tag=f"psdsb{bk % 2}")
                nc.scalar.copy(psd_sb[:],
                               psd_ps[:, bk * BANK:(bk + 1) * BANK])
                eng = nc.sync if bk % 2 == 0 else nc.scalar
                eng.dma_start(psd_d[:, bk * BANK:(bk + 1) * BANK], psd_sb[:])
    nc.compile()
    return nc


FB = 9  # f-blocks of 64 (last block holds only f=512)
TH = 4  # t-tiles per half


def _build_apply_kernel_v5():
    """Apply from the original f32 data; output lands f-major.

    Per T-half (4 t-tiles): load + cast each tile, then per f-block-pair:
    PE-transpose the needed [128t, 128(fl,ri)] chunks to f-major, and run
    rotation matmuls with wt[:, fb, c, :] as the stationary operand
    (reloaded once per (fb, c), reused across the 4 t-tiles), accumulating
    over channels in PSUM.  out[(f,ri), t] is written f-major; the host
    transposes.  f=512 handled via a tiny PE-transpose + K=16 matmul.
    """
    import concourse.bacc as bacc
    import concourse.mybir as mybir
    from concourse.tile import TileContext
    from concourse import masks as cmasks

    nc = bacc.Bacc(None, target_bir_lowering=False)
    fp = mybir.dt.float32
    bh = mybir.dt.bfloat16
    data_d = nc.dram_tensor("data", [T, C, F, 2], fp, kind="ExternalInput")
    wt_d = nc.dram_tensor("wt", [P, FB, C, P], bh, kind="ExternalInput")
    out_d = nc.dram_tensor("out", [2 * F, T], fp, kind="ExternalOutput")

    with TileContext(nc) as tc:
        with tc.tile_pool(name="const", bufs=1) as cpool, \
             tc.tile_pool(name="dt", bufs=2) as dtpool, \
             tc.tile_pool(name="bfc", bufs=TH + 1) as bfpool, \
             tc.tile_pool(name="xt", bufs=4) as xpool, \
             tc.tile_pool(name="x5", bufs=2) as x5pool, \
             tc.tile_pool(name="tpg", bufs=2, space="PSUM") as tgpool, \
             tc.tile_pool(name="tp5", bufs=1, space="PSUM") as t5pool, \
             tc.tile_pool(name="ps", bufs=2, space="PSUM") as pspool, \
             tc.tile_pool(name="oc", bufs=2, space="PSUM") as ocpool, \
             tc.tile_pool(name="eo", bufs=3) as epool, \
             tc.tile_pool(name="e5", bufs=1) as e5pool:
            identb = cpool.tile([P, P], bh)
            cmasks.make_identity(nc, identb[:])
            wt = cpool.tile([P, FB, C, P], bh, tag="wt")
            nc.sync.dma_start(wt[:], wt_d[:])

            eo5 = e5pool.tile([2, T], fp, tag="eo5")
            for half in range(2):
                tts = list(range(half * TH, (half + 1) * TH))
                bfcs = {}
                for it in tts:
                    dt = dtpool.tile([P, C, F, 2], fp, tag="dt", name="dt")
                    eng = nc.sync if it % 2 == 0 else nc.scalar
                    eng.dma_start(dt[:], data_d[it * P:(it + 1) * P])
                    bfc = bfpool.tile([P, C, F, 2], bh, tag="bfc",
                                      name="bfc")
                    nc.scalar.copy(bfc[:, 0:4], dt[:, 0:4])
                    nc.vector.tensor_copy(bfc[:, 4:8], dt[:, 4:8])
                    bfcs[it] = bfc
                    # f=512, this tile: transpose [128t, 16(c,ri)] -> K=16
                    tp5 = t5pool.tile([16, P], bh, tag="tp5", name="tp5")
                    nc.tensor.transpose(tp5[:, :], bfc[:, :, 512, :],
                                        identb[:])
                    x5 = x5pool.tile([16, P], bh, tag="x5", name="x5")
                    nc.vector.tensor_copy(x5[:], tp5[:])
                    oc = ocpool.tile([2, P], fp, tag="oc", name="oc")
                    nc.tensor.matmul(oc[:], wt[0:16, 8, 0, 0:2], x5[:],
                                     start=True, stop=True)
                    nc.vector.tensor_copy(eo5[:, it * P:(it + 1) * P], oc[:])
                for fb in range(8):
                    ps = pspool.tile([P, TH * P], fp, tag="ps", name="ps")
                    for cp in range(C // 2):
                        # stage transposes for channels (2cp, 2cp+1) x 4
                        # t-tiles as one [128, 4, 128] psum group each, so
                        # the psum->sbuf copy is one big instruction
                        xts = []
                        for ci in range(2):
                            c = 2 * cp + ci
                            tpg = tgpool.tile([P, TH, P], bh, tag="tpg",
                                              name="tpg")
                            for k, it in enumerate(tts):
                                # start=True clears the whole bank ->
                                # only the first transpose per tpg sets it
                                nc.tensor.matmul(
                                    tpg[:, k, :],
                                    bfcs[it][:, c, 64 * fb:64 * (fb + 1), :],
                                    identb[:], is_transpose=True,
                                    start=(k == 0), stop=(k == TH - 1),
                                    skip_group_check=True)
                            xt = xpool.tile([P, TH, P], bh, tag="xt",
                                            name="xt")
                            if (fb + ci) % 2 == 0:
                                nc.vector.tensor_copy(xt[:], tpg[:])
                            else:
                                nc.scalar.copy(xt[:], tpg[:])
                            xts.append(xt)
                        for ci in range(2):
                            c = 2 * cp + ci
                            for k in range(TH):
                                nc.tensor.matmul(
                                    ps[:, k * P:(k + 1) * P],
                                    wt[:, fb, c, :], xts[ci][:, k, :],
                                    start=(c == 0 and k == 0),
                                    stop=(c == C - 1 and k == TH - 1),
                                    skip_group_check=True)
                    eo = epool.tile([P, TH * P], fp, tag="eo", name="eo")
                    if fb % 2 == 0:
                        nc.vector.tensor_copy(eo[:], ps[:])
                    else:
                        nc.scalar.copy(eo[:], ps[:])
                    eng = nc.sync if fb % 2 == 0 else nc.scalar
                    eng.dma_start(
                        out_d[128 * fb:128 * (fb + 1),
                              half * TH * P:(half + 1) * TH * P], eo[:])
            nc.sync.dma_start(out_d[2 * F - 2:2 * F, :], eo5[:])
    nc.compile()
    return nc


def _build_wt(bf):
    """bf: (F, C) complex64 -> Wt (128, FB, C, 128) bf16 (block-diag rot).

    fb<8: W[fb, c, (fl,ri), (fl2,ri2)] block-diagonal per fl.
    fb=8 slot [0:16, 8, 0, 0:2]: rows (c,ri), cols ri2 for f=512.
    """
    import ml_dtypes
    wa = bf.real.astype(np.float32)   # (F, C)
    wb = bf.imag.astype(np.float32)
    W = np.zeros((FB, C, P, P), np.float32)
    fl = np.arange(64)
    for fb in range(8):
        f = 64 * fb + fl
        for c in range(C):
            W[fb, c, 2 * fl, 2 * fl] = wa[f, c]
            W[fb, c, 2 * fl + 1, 2 * fl] = wb[f, c]
            W[fb, c, 2 * fl, 2 * fl + 1] = -wb[f, c]
            W[fb, c, 2 * fl + 1, 2 * fl + 1] = wa[f, c]
    for c in range(C):
        W[8, 0, 2 * c, 0] = wa[512, c]
        W[8, 0, 2 * c + 1, 0] = wb[512, c]
        W[8, 0, 2 * c, 1] = -wb[512, c]
        W[8, 0, 2 * c + 1, 1] = wa[512, c]
    return np.ascontiguousarray(
        W.transpose(2, 0, 1, 3)).astype(ml_dtypes.bfloat16)


def _decode_psd(raw, msum):
    """raw: (B,128,2560), msum: (B,128,10) -> psd_s, psd_n (B,F,C,C) c64."""
    nb = raw.shape[0]
    slots = np.empty((nb, 32, F, 16), np.float32)
    g = np.arange(F) // 4
    j = np.arange(F) % 4
    off = (g // 32) * BANK + (g % 32) * PSUM_W
    for jj in range(4):
        sel = j == jj
        cols = off[sel][:, None] + np.arange(16)[None]
        slots[:, :, sel, :] = raw[:, 32 * jj:32 * (jj + 1), :][
            :, :, cols.reshape(-1)].reshape(nb, 32, sel.sum(), 16)
    ms_sum = np.empty((nb, 2, F), np.float32)
    for mk in range(2):
        for ft, (f0, pf) in enumerate(F_TILES):
            ms_sum[:, mk, f0:f0 + pf] = msum[:, :pf, mk * 5 + ft]
    psds = []
    for mk in range(2):
        r = slots[:, 16 * mk:16 * mk + 16:2]     # (B, 8c, F, 16)
        i = slots[:, 16 * mk + 1:16 * mk + 16:2]
        re = r[..., 0::2] + i[..., 1::2]          # (B, c, F, e)
        im = i[..., 0::2] - r[..., 1::2]
        psd = (re + 1j * im).astype(np.complex64).transpose(0, 2, 1, 3)  # (B,F,c,e)
        scale = 1.0 / (ms_sum[:, mk] + C * EPS_PSD)
        psds.append(psd * scale[:, :, None, None].astype(np.complex64))
    return psds[0], psds[1]


def _middle(psd_s, psd_n, W_psd, b_psd, w_gvec, b_gvec):
    eye = np.eye(C, dtype=bool)
    psd = np.where(eye[None, None], np.complex64(0), psd_s)
    psd = np.swapaxes(psd.sum(axis=-1) / (C - 1), -1, -2)
    psd_feat = np.abs(psd).astype(np.float32)
    e = np.tanh(psd_feat @ W_psd + b_psd) @ w_gvec + b_gvec[0]
    e = SCALING * e
    e = e - e.max(axis=-1, keepdims=True)
    ex = np.exp(e)
    u = (ex / ex.sum(axis=-1, keepdims=True)).astype(np.float32)
    psd_n_reg = psd_n + (EPS_MVDR * np.eye(C)).astype(np.complex64)
    num = np.matmul(np.linalg.inv(psd_n_reg), psd_s)
    trace = np.einsum('bfcc->bf', num)
    ws = num / (trace[..., None, None] + EPS_MVDR)
    bf = np.einsum('bfec,bc->bfe', ws, u.astype(ws.dtype))  # (B,F,C)
    return bf


_RES = {"t_psd": None, "t_apply": None}


def _run(nc, in_maps, trace=False):
    from concourse.bass_utils import run_bass_kernel_spmd
    return run_bass_kernel_spmd(nc, in_maps, core_ids=list(range(B)),
                                trace=trace)


def _run_timed(nc, in_maps, trace):
    """NTFF profiling corrupts the non-profiled cores, so when tracing is
    requested run once traced (timing) and once clean (outputs)."""
    if not trace:
        return _run(nc, in_maps), None
    rt = _run(nc, in_maps, trace=True)
    rc = _run(nc, in_maps, trace=False)
    return rc, rt.exec_time_ns


def _device_pipeline(data_ri, mask_speech, mask_noise,
                     W_psd, b_psd, w_gvec, b_gvec):
    trace = bool(os.environ.get("BEAM_TRACE"))
    nc1 = _build_psd_kernel()
    in1 = [{"data": data_ri[b],
            "ms": np.ascontiguousarray(mask_speech[b], np.float32),
            "mn": np.ascontiguousarray(mask_noise[b], np.float32)}
           for b in range(B)]
    r1, t1 = _run_timed(nc1, in1, trace)
    _RES["t_psd"] = t1
    raw = np.stack([r["psd_raw"] for r in r1.results])
    msum = np.stack([r["msum"] for r in r1.results])

    psd_s, psd_n = _decode_psd(raw, msum)
    bf = _middle(psd_s, psd_n, W_psd, b_psd, w_gvec, b_gvec)

    nc2 = _build_apply_kernel_v5()
    in2 = [{"data": data_ri[b], "wt": _build_wt(bf[b])} for b in range(B)]
    r2, t2 = _run_timed(nc2, in2, trace)
    _RES["t_apply"] = t2
    # out: (2F, T) f-major -> (T, F, 2)
    outs = np.stack([r["out"] for r in r2.results])      # (B, 2F, T)
    return np.ascontiguousarray(
        outs.reshape(B, F, 2, T).transpose(0, 3, 1, 2))


def _numpy_pipeline(data_ri, mask_speech, mask_noise,
                    W_psd, b_psd, w_gvec, b_gvec):
    data = data_ri[..., 0] + 1j * data_ri[..., 1]
    x = np.ascontiguousarray(np.transpose(data, (0, 3, 2, 1)))  # (B,F,C,T)
    psds = []
    for mask in (mask_speech, mask_noise):
        m = np.clip(mask, EPS_MASK, None).mean(axis=-2)
        m = m / (m.sum(axis=-1, keepdims=True) + EPS_PSD)
        xw = x * m[:, :, None, :].astype(x.dtype)
        psds.append(np.matmul(xw, np.conj(np.swapaxes(x, -1, -2))))
    bf = _middle(psds[0], psds[1], W_psd, b_psd, w_gvec, b_gvec)
    enh = np.einsum('bfc,bfct->bft', np.conj(bf), x)
    enh = np.swapaxes(enh, -1, -2)
    return np.stack([enh.real, enh.imag], axis=-1).astype(np.float32)


def kernel(data_ri, mask_speech, mask_noise, W_psd, b_psd, w_gvec, b_gvec,
           ilens):
    data_ri = np.ascontiguousarray(data_ri, dtype=np.float32)
    mask_speech = np.asarray(mask_speech, np.float32)
    mask_noise = np.asarray(mask_noise, np.float32)
    W_psd = np.asarray(W_psd, np.float32)
    b_psd = np.asarray(b_psd, np.float32)
    w_gvec = np.asarray(w_gvec, np.float32)
    b_gvec = np.asarray(b_gvec, np.float32)
    if os.environ.get("BEAM_NO_DEVICE"):
        return _numpy_pipeline(data_ri, mask_speech, mask_noise,
                               W_psd, b_psd, w_gvec, b_gvec)
    import signal
    old = None
    try:
        if hasattr(signal, "SIGALRM"):
            def _timeout(signum, frame):
                raise TimeoutError("device pipeline watchdog")
            old = signal.signal(signal.SIGALRM, _timeout)
            signal.alarm(900)
        return _device_pipeline(data_ri, mask_speech, mask_noise,
                                W_psd, b_psd, w_gvec, b_gvec)
    except Exception as exc:  # device unavailable -> still return correctly
        sys.stderr.write(f"device pipeline failed ({exc!r}); numpy fallback\n")
        return _numpy_pipeline(data_ri, mask_speech, mask_noise,
                               W_psd, b_psd, w_gvec, b_gvec)
    finally:
        if old is not None:
            signal.alarm(0)
            signal.signal(signal.SIGALRM, old)
